# revision 22
# baseline (speedup 1.0000x reference)
"""Causal attention head (B=4, T=4096, D=1024, H=64) on 8 TRN2 NeuronCores.

Sharding: 2 cores per batch element. Within a batch, core role r in {0,1}
owns the interleaved query rows {256*v + 2*i + r : v in [0,16), i in [0,128)}.
Every core runs an IDENTICAL instruction stream (SPMD-uniform). The role
offset r lives entirely in host-staged data: the per-core x^T has its
columns pair-swapped for r=1 (token c^r at column c), so the core's own
query rows sit at even columns {256*v + 2*i} for both roles and the causal
masks (which absorb the within-pair order) are per-core input data.

Per-core device program (v2 — DMA priority + engine rebalance):
  - constants split in two blobs on the two HWDGE queues: cbA = Wk|Wv on
    sync, cbB = Wq | identity(x2) | masks on scalar; x^T streamed as
    HALF-slices alternating across both queues so arrival order matches
    consumption order (slice s fully landed ~2.4us apart)
  - PE warmup matmuls fill the program-load + DMA ramp
  - K^T/V^T projection with lhsT=[Wk|Wv]; Q^T projection column-tiled:
    two concurrent M=64 matmuls (query halves) on disjoint PE col groups
  - V^T -> V via PAIRED PE transposes on disjoint row groups (even tiles
    read V^T at partitions 64:128, odd tiles read a V^T copy at 0:64)
  - flash-style attention over four 512-query-column groups; S^T strips in
    PSUM, exp on ScalarE (the pacing engine, kept saturated), causal mask
    multiplies split across GpSimd (strip A) and VectorE (strip B) so they
    never pace, ctx^T accumulated per group with a fused ones-column
    denominator; late projections (kv5-7, q3) and late V transposes are
    deferred into groups 2-3 to fill PE slack under the exp stream
  - group-3 epilogue split in two so the last output DMA starts early
Host side: shard/cast/permute inputs, gather + re-interleave outputs.
"""

import numpy as np
import ml_dtypes

import concourse.tile as tile
import concourse.mybir as mybir
from concourse import bacc
from concourse.bass_utils import run_bass_kernel_spmd

BF16 = ml_dtypes.bfloat16
F32 = np.float32

B, T, D, H = 4, 4096, 1024, 64
TL = 2048          # local query columns per core
N_CORES = 8
NKT = T // 128     # 32 key tiles
NV = TL // 128     # 16 virtual query tiles
DCH = D // 128     # 8 contraction chunks
DT_BF = mybir.dt.bfloat16
DT_F32 = mybir.dt.float32
EXP = mybir.ActivationFunctionType.Exp
MUL = mybir.AluOpType.mult

N_WARM = 120        # scratch matmuls to warm the PE during the DMA wait

# cbA (sync queue): Wk|Wv
CA_WKV = 0                 # [d, 128] -> 1024 cols
CA_COLS = 1024
# cbB (scalar queue): Wq | identity (both halves) | diagonal masks A|B
CB_WQ = 0                  # [d, 128] (64 real + 64 zero pad, FWL) -> 1024 cols
CB_ID = 1024               # [128, 64]: rows 0:64 = I64, rows 64:128 = I64
CB_MASK = 1088             # [128, 256] diagonal masks A|B
CB_COLS = 1344


def _build():
    nc = bacc.Bacc("TRN2", target_bir_lowering=False, debug=False,
                   num_devices=N_CORES)

    xt = nc.dram_tensor("xt", [128, DCH * T], DT_BF, kind="ExternalInput").ap()
    cba = nc.dram_tensor("cba", [128, CA_COLS], DT_BF, kind="ExternalInput").ap()
    cbb = nc.dram_tensor("cbb", [128, CB_COLS], DT_BF, kind="ExternalInput").ap()
    y = nc.dram_tensor("y", [65, TL], DT_F32, kind="ExternalOutput").ap()

    with tile.TileContext(nc) as tc:
        _body(nc, tc, xt, cba, cbb, y)

    nc.compile()
    return nc


def _body(nc, tc, xt, cba, cbb, y):
    from contextlib import ExitStack

    es = ExitStack()
    with es:
        pp = es.enter_context(tc.tile_pool(name="persist", bufs=1))
        xt_sb = pp.tile([128, DCH * T], DT_BF)
        cba_sb = pp.tile([128, CA_COLS], DT_BF)
        cbb_sb = pp.tile([128, CB_COLS], DT_BF)
        kvT_sb = pp.tile([128, T], DT_BF)       # rows 0:64 = K^T, 64:128 = V^T
        kvT2_sb = pp.tile([128, T], DT_BF)      # rows 0:64 = V^T copy, 64:128 = K^T copy
        qT_sb = pp.tile([64, TL], DT_BF)
        qT2_sb = pp.tile([128, TL], DT_BF)      # rows 64:128 = Q^T copy
        vones_sb = pp.tile([128, NKT * 128], DT_BF)  # V tiles + ones col + pad
        warm_sb = pp.tile([128, 64], DT_BF)

        # ---- input DMAs: priority-ordered across the two HWDGE queues ----
        # sync:   [cbA(Wkv), x0a, x1a, ..., x7a, (y outs)]
        # scalar: [cbB(Wq|id|masks), x0b, x1b, ..., x7b]
        # Engines round-robin both queues, so slice s (both halves) lands
        # ~2.4us after slice s-1 and the weight blobs land first.
        # All xt moves as HALF-slices (uniform 4KB descriptors -> the two
        # queues round-robin bytes 1:1). The scalar queue gets ONLY 4 pushes
        # (cb blobs + slice-0/1 b-halves) so the scalar engine is never
        # blocked on DMA-semaphore reuse and the exp stream starts early.
        # Everything else rides the sync queue in need-order.
        nc.scalar.dma_start(cba_sb[:], cba[:])
        nc.scalar.dma_start(cbb_sb[:], cbb[:])
        # slice 0 moves as QUARTERS: quarter qi exactly feeds kv-quarter qi,
        # so the first projections start as soon as the first 0.26MB lands.
        for qi in range(4):
            nc.sync.dma_start(xt_sb[:, qi * 1024:(qi + 1) * 1024],
                              xt[:, qi * 1024:(qi + 1) * 1024])
        for s in range(1, 8):
            lo = s * 4096
            nc.sync.dma_start(xt_sb[:, lo:lo + 2048], xt[:, lo:lo + 2048])
            nc.sync.dma_start(xt_sb[:, lo + 2048:lo + 4096],
                              xt[:, lo + 2048:lo + 4096])

        nc.gpsimd.memset(warm_sb[:], 0.0)
        # vones: zero pad cols, ones at col 64 of each 128-wide tile
        nc.vector.memset(vones_sb[:], 0.0)
        vones_v = vones_sb.rearrange("p (t k) -> p t k", k=128)
        nc.vector.memset(vones_v[:, :, 64:65], 1.0)

        # strided view for Q: own query rows at even columns
        # [p, s(8), d(8), vh(2), i(128), par(2)]; par=0 selects own rows
        xt_q = xt_sb.rearrange("p (s d vh i q) -> p s d vh i q",
                               s=8, d=DCH, vh=2, i=128, q=2)

        psum_proj = es.enter_context(
            tc.tile_pool(name="psum_proj", bufs=2, space="PSUM"))
        psum_vt = es.enter_context(
            tc.tile_pool(name="psum_vt", bufs=1, space="PSUM"))

        proj_tiles = {}   # live PSUM tile per in-flight projection

        def emit_q_q(s, qi):
            """Quarter qi of q-slice s projection (d chunks 2qi, 2qi+1)."""
            if qi == 0:
                proj_tiles["q", s] = psum_proj.tile(
                    [128, 512], DT_F32, name=f"pq{s}", tag="proj")
            pq = proj_tiles["q", s]
            for d in range(2 * qi, 2 * qi + 2):
                # lhsT is the 128-col padded Wq chunk (FWL fast path);
                # output rows 64:128 are zeros and never read.
                nc.tensor.matmul(
                    pq[:],
                    lhsT=cbb_sb[:, CB_WQ + d * 128: CB_WQ + (d + 1) * 128],
                    rhs=xt_q[:, 2 * s:2 * s + 2, d, :, :, 0:1],
                    start=(d == 0), stop=(d == DCH - 1))
            if qi == 3:
                sl = slice(s * 512, (s + 1) * 512)
                nc.vector.tensor_copy(qT_sb[:, sl], pq[0:64, :])
                nc.vector.tensor_copy(qT2_sb[64:128, sl], pq[0:64, :])
                del proj_tiles["q", s]

        def emit_kv_q(s, qi):
            """Quarter qi of kv-slice s projection + dup copies at qi==3."""
            if qi == 0:
                proj_tiles["kv", s] = psum_proj.tile(
                    [128, 512], DT_F32, name=f"pkv{s}", tag="proj")
            pkv = proj_tiles["kv", s]
            for d in range(2 * qi, 2 * qi + 2):
                nc.tensor.matmul(
                    pkv[:],
                    lhsT=cba_sb[:, CA_WKV + d * 128: CA_WKV + (d + 1) * 128],
                    rhs=xt_sb[:, s * 4096 + d * 512: s * 4096 + d * 512 + 512],
                    start=(d == 0), stop=(d == DCH - 1))
            if qi == 3:
                sl = slice(s * 512, (s + 1) * 512)
                nc.vector.tensor_copy(kvT_sb[:, sl], pkv[:])
                nc.vector.tensor_copy(kvT2_sb[64:128, sl], pkv[0:64, :])
                del proj_tiles["kv", s]

        def emit_vt(t):
            """V transpose for key tile t."""
            pv = psum_vt.tile([128, 64], DT_BF, name=f"pv{t}", tag="pv")
            nc.tensor.transpose(pv[:],
                                kvT_sb[64:128, t * 128:(t + 1) * 128],
                                cbb_sb[64:128, CB_ID: CB_ID + 64])
            nc.vector.tensor_copy(vones_sb[:, t * 128: t * 128 + 64], pv[:])

        # ---- PE warmup during program load + DMA ramp ----
        wps = psum_proj.tile([64, 64], DT_F32, name="warm", tag="proj")
        for _ in range(N_WARM):
            nc.tensor.matmul(wps[:], lhsT=warm_sb[:, 0:64],
                             rhs=warm_sb[:, 0:64], start=True, stop=True)

        # ramp: kv0 needs xt slice 0; vt0-2 need kv0; q0 needs xt 0-1.
        for qi in range(4):
            emit_kv_q(0, qi)
        for qi in range(4):
            emit_q_q(0, qi)
        emit_vt(0)
        emit_vt(1)
        emit_vt(2)

        # ---- attention: four 512-query-column groups ----
        # group g covers local q cols [512g, 512g+512), key tiles j=0..8g+7.
        # Fillers (later projections + V transposes) are interleaved INTO the
        # strip loops so they run in PE slack while ScalarE paces the exp
        # stream; data-gated work (q_{g+1}) sits at group ends.
        KV, Q, VT = emit_kv_q, emit_q_q, emit_vt
        interleave = {
            (0, 1): [(KV, 1, 0), (KV, 1, 1), (VT, 3)],
            (0, 2): [(KV, 1, 2), (KV, 1, 3), (VT, 4)],
            (1, 1): [(KV, 2, 2)], (1, 2): [(KV, 2, 3)], (1, 3): [(KV, 3, 0)],
            (1, 4): [(KV, 3, 1), (VT, 8)], (1, 5): [(KV, 3, 2), (KV, 3, 3)],
            (1, 6): [(VT, 9), (VT, 10)], (1, 7): [(VT, 11), (VT, 12)],
            (2, 1): [(KV, 4, 0)], (2, 2): [(KV, 4, 1)], (2, 3): [(KV, 4, 2)],
            (2, 4): [(KV, 4, 3), (Q, 3, 0)], (2, 5): [(KV, 5, 0), (Q, 3, 1)],
            (2, 6): [(KV, 5, 1), (Q, 3, 2)], (2, 7): [(KV, 5, 2), (Q, 3, 3)],
            (2, 8): [(KV, 5, 3), (VT, 16)],
            (2, 9): [(VT, 17), (VT, 18), (KV, 6, 0)],
            (2, 10): [(VT, 19), (VT, 20), (KV, 6, 1)],
            (2, 11): [(VT, 21), (VT, 22), (KV, 6, 2)],
            (3, 1): [(KV, 6, 3)], (3, 2): [(KV, 7, 0)], (3, 3): [(KV, 7, 1)],
            (3, 4): [(KV, 7, 2)], (3, 5): [(KV, 7, 3)],
            (3, 6): [(VT, 24)], (3, 7): [(VT, 25)], (3, 8): [(VT, 26)],
            (3, 9): [(VT, 27)], (3, 10): [(VT, 28)], (3, 11): [(VT, 29)],
            (3, 12): [(VT, 30)], (3, 13): [(VT, 31)],
        }
        post_group = {
            0: [(VT, 5), (VT, 6), (VT, 7), (KV, 2, 0), (KV, 2, 1),
                (Q, 1, 0), (Q, 1, 1), (Q, 1, 2), (Q, 1, 3)],
            1: [(VT, 13), (VT, 14), (VT, 15),
                (Q, 2, 0), (Q, 2, 1), (Q, 2, 2), (Q, 2, 3)],
            2: [(VT, 23)],
            3: [],
        }

        with tc.tile_pool(name="psum_ctx", bufs=1, space="PSUM") as pctx, \
             tc.tile_pool(name="psum_strip", bufs=2, space="PSUM") as pstrip, \
             tc.tile_pool(name="pT", bufs=6) as ppT, \
             tc.tile_pool(name="ep_sb", bufs=2) as pes:
            for g in range(4):
                base = 512 * g
                jmax = 8 * g + 7
                ctx_ps = pctx.tile([128, 512], DT_F32, name=f"ctx{g}", tag="ctx")
                pending = []   # deferred ctx matmuls, depth-2 pipeline

                def flush_ctx():
                    # lhsT is the full 128-col padded V tile so the weight
                    # load takes the FWL fast path (cols 65:128 are zero).
                    fpt, fc_lo, fw, fja, fjb, fboff = pending.pop(0)
                    nc.tensor.matmul(
                        ctx_ps[:, fc_lo - base: 512],
                        lhsT=vones_sb[:, fja * 128: fja * 128 + 128],
                        rhs=fpt[:, 512 - fw: 512],
                        start=(fja == 0), stop=False)
                    nc.tensor.matmul(
                        ctx_ps[:, fc_lo - base + fboff: 512],
                        lhsT=vones_sb[:, fjb * 128: fjb * 128 + 128],
                        rhs=fpt[:, 512 + fboff: 512 + fw],
                        start=False, stop=(fjb == jmax))

                # strip pairs: p covers key tiles jA=2p (PE rows 0:63) and
                # jB=2p+1 (PE rows 64:127), concurrent on disjoint row
                # groups. Strip A sits at [512-w,512), B at [512,512+w) so
                # each matmul output stays inside one PSUM bank.
                for p in range(4 * g + 4):
                    for unit in interleave.get((g, p), ()):
                        unit[0](*unit[1:])
                    jA, jB = 2 * p, 2 * p + 1
                    q0 = 128 * p
                    c_lo = max(q0, base)
                    w = base + 512 - c_lo
                    # on diagonal pairs the first 64 cols of strip B are
                    # FULLY masked: skip them in the B matmul, mask and ctx
                    # (the exp'd garbage there is never read).
                    boff = 64 if c_lo == q0 else 0
                    ps = pstrip.tile([128, 1024], DT_F32,
                                     name=f"ps{g}_{p}", tag="ps")
                    nc.tensor.matmul(
                        ps[:, 512 - w: 512],
                        lhsT=kvT_sb[0:64, jA * 128:(jA + 1) * 128],
                        rhs=qT_sb[:, c_lo: base + 512],
                        start=True, stop=True)
                    nc.tensor.matmul(
                        ps[:, 512: 512 + w],
                        lhsT=kvT2_sb[64:128, jB * 128:(jB + 1) * 128],
                        rhs=qT2_sb[64:128, c_lo: base + 512],
                        start=True, stop=True)
                    # ctx matmuls run two pairs behind the scores so the
                    # exp->ctx latency fully hides
                    if len(pending) == 2:
                        flush_ctx()
                    pt = ppT.tile([128, 1024], DT_BF, name=f"pt{g}_{p}", tag="pt")
                    nc.scalar.activation(pt[:, 512 - w: 512 + w],
                                         ps[:, 512 - w: 512 + w],
                                         EXP, bias=0.0, scale=0.125)
                    if c_lo == q0:  # diagonal pair: causal masks, 128 cols
                        # each; split across GpSimd/VectorE so they never pace
                        nc.gpsimd.tensor_tensor(
                            pt[:, 512 - w: 512 - w + 128],
                            pt[:, 512 - w: 512 - w + 128],
                            cbb_sb[:, CB_MASK: CB_MASK + 128], MUL)
                        nc.vector.tensor_tensor(
                            pt[:, 576: 640], pt[:, 576: 640],
                            cbb_sb[:, CB_MASK + 192: CB_MASK + 256], MUL)
                    pending.append((pt, c_lo, w, jA, jB, boff))
                for unit in post_group.get(g, ()):
                    unit[0](*unit[1:])
                while pending:
                    flush_ctx()
                # group epilogue: raw [num;den]^T out; divide on host
                cs = pes.tile([65, 512], DT_F32, name=f"cs{g}", tag="cs")
                nc.vector.tensor_copy(cs[:], ctx_ps[0:65, :])
                nc.sync.dma_start(y[:, base: base + 512], cs[:])


_ROW_IDX = [np.array([256 * v + 2 * i + r for v in range(NV) for i in range(128)])
            for r in range(2)]


def _host_prep(inputs):
    x = np.asarray(inputs["x"], dtype=F32)
    Wk = np.asarray(inputs["Wk"], dtype=F32)
    Wq = np.asarray(inputs["Wq"], dtype=F32)
    Wv = np.asarray(inputs["Wv"], dtype=F32)

    # [d*128+p, t] -> [p, d*t] views for the constant blobs
    wq_v = np.zeros((128, DCH * 128), dtype=F32)
    wq_v[:, np.arange(DCH * 128).reshape(DCH, 128)[:, :H].ravel()] = (
        Wq.reshape(DCH, 128, H).transpose(1, 0, 2).reshape(128, DCH * H))
    wkv = np.concatenate([Wk, Wv], axis=1)
    wkv_v = np.ascontiguousarray(
        wkv.reshape(DCH, 128, 128).transpose(1, 0, 2).reshape(128, DCH * 128))
    identb = np.zeros((128, 64), dtype=F32)
    identb[0:64, :] = np.eye(64, dtype=F32)
    identb[64:128, :] = np.eye(64, dtype=F32)

    kk = np.arange(128)[:, None]
    ii = np.arange(128)[None, :]
    in_maps = []
    for c in range(N_CORES):
        b, r = c // 2, c % 2
        # pair-swap permutation: token c^r at column c; slice-major
        # device layout [p, s(8), d(8), c(512)]
        perm = np.arange(T) ^ r
        xp = x[b][perm]                      # [T, D]
        xt_np = np.ascontiguousarray(
            xp.reshape(8, 512, DCH, 128).transpose(3, 0, 2, 1)
            .reshape(128, DCH * T)).astype(BF16)
        tok = kk ^ r                      # within-tile token offset at row k
        maskA = (tok <= 2 * ii + r)
        maskB = (tok + 128 <= 2 * ii + r)
        cba_np = wkv_v.astype(BF16)
        cbb_np = np.zeros((128, CB_COLS), dtype=F32)
        cbb_np[:, CB_WQ:CB_WQ + DCH * 128] = wq_v
        cbb_np[:, CB_ID:CB_ID + 64] = identb
        cbb_np[:, CB_MASK:CB_MASK + 128] = maskA
        cbb_np[:, CB_MASK + 128:CB_MASK + 256] = maskB
        in_maps.append(dict(xt=xt_np, cba=cba_np, cbb=cbb_np.astype(BF16)))
    return in_maps


def _gather(results):
    out = np.zeros((B, T, H), dtype=F32)
    for c in range(N_CORES):
        b, r = c // 2, c % 2
        yc = results[c]["y"]  # [65, TL]: rows 0:64 = ctx^T, row 64 = denom
        out[b, _ROW_IDX[r]] = (yc[:64, :] / yc[64:65, :]).T
    return out


_NC_CACHE = []


def _execute(inputs, trace=False):
    if not _NC_CACHE:
        _NC_CACHE.append(_build())
    nc = _NC_CACHE[0]
    in_maps = _host_prep(inputs)
    res = run_bass_kernel_spmd(nc, in_maps, core_ids=list(range(N_CORES)),
                               trace=trace)
    return _gather(res.results), res


def kernel(**inputs):
    out, _ = _execute(inputs, trace=False)
    return out
